# revision 20
# baseline (speedup 1.0000x reference)
"""CapsNet dynamic-routing layer on 8 Trainium2 NeuronCores.

reference: u_hat = einsum('jide,bie->bjid', W, x); 3 rounds of
softmax(cc, axis=j) -> s = sum_i c*u_hat -> v = squash(s) -> cc += u_hat.v

Strategy (pure data parallel, batch 256 -> 32 per core, W replicated):
  - u_hat produced on the PE with block-diagonal x stationaries
    [K=32=(i4,e8), M=128=(i4,b32)] x dense W slices [K=32, N=160=(j,d)],
    4 row-tiles (tile_position) in flight -> PSUM [(i4,b32), (j,d)],
    evacuated (f32->fp16, DVE/ACT split) to SBUF u[128, h=288, j, d],
    i = 4*h + i4.
  - round 0: c uniform -> s0 = (1/10) sum_i u_hat via dense K=128 matmuls
    accumulating 72 tiles into one PSUM bank.
  - cc-dot (cc += u.v): DVE mul (2x) + one halving tree level, then the
    d-reduction and the cross-partition i4-reduction ride the PE as
    accumulating matmuls with an identity stationary into PSUM chunks.
  - softmax consumes cc PSUM chunks directly (exp on ACT); c is stored as
    duplicated fp16 pairs (c_pk[..., 2]) so the weighted-sum multiply
    broadcasts c over d via packed 32-bit reads at DVE 2x.
  - weighted sum: DVE mul (2x) + per-h accumulating PE matmuls with a
    0/1 replication stationary -> s in PSUM [32, (j,d)]; squash in f32.
All host-side work is layout marshaling only (scatter/transpose/cast).
"""

import sys
from contextlib import ExitStack

import numpy as np

sys.path.insert(0, "/opt/trn_rl_repo")

import concourse.bacc as bacc  # noqa: E402
import concourse.tile as tile  # noqa: E402
from concourse import mybir  # noqa: E402
from concourse.bass_utils import run_bass_kernel_spmd  # noqa: E402

B, J, I, D, E = 256, 10, 1152, 16, 8
NCORES = 8
BL = B // NCORES            # 32 local batch per core
G = I // 16                 # 72 groups of 16 input caps
GH = G // 2                 # 36 groups per DMA half
H = I // 4                  # 288 (h = 4*g + r), i = 4*h + c4
JD = J * D                  # 160
EPS = 1e-7
HC = 48                     # h-chunk for routing passes
NQ = H // HC                # 6 chunks

f32 = mybir.dt.float32
f16 = mybir.dt.float16
ADD = mybir.AluOpType.add
MUL = mybir.AluOpType.mult
AX = mybir.AxisListType.X


def _build_bass():
    nc = bacc.Bacc("TRN2", target_bir_lowering=False)

    xd = nc.dram_tensor("xd", [128, G, BL], f16, kind="ExternalInput").ap()
    xblk = nc.dram_tensor("xblk", [128, G, 128], f16,
                          kind="ExternalInput").ap()
    wd = nc.dram_tensor("wd", [128, G, JD], f16, kind="ExternalInput").ap()
    rep4 = nc.dram_tensor("rep4", [BL, 128], f16, kind="ExternalInput").ap()
    rep4t = nc.dram_tensor("rep4t", [128, BL], f16, kind="ExternalInput").ap()
    eye = nc.dram_tensor("eye", [128, 128], f16, kind="ExternalInput").ap()
    vout = nc.dram_tensor("vout", [BL, JD], f32, kind="ExternalOutput").ap()

    with tile.TileContext(nc) as tc, ExitStack() as ctx, \
            nc.allow_low_precision("fp16 intermediates within tolerance"):
        singles = ctx.enter_context(tc.tile_pool(name="singles", bufs=1))
        halves = ctx.enter_context(tc.tile_pool(name="halves", bufs=2))
        work = ctx.enter_context(tc.tile_pool(name="work", bufs=2))
        smalls = ctx.enter_context(tc.tile_pool(name="smalls", bufs=1))
        psum_prod = ctx.enter_context(
            tc.tile_pool(name="psum_prod", bufs=2, space="PSUM"))
        psum_cc = ctx.enter_context(
            tc.tile_pool(name="psum_cc", bufs=2, space="PSUM"))
        psum_misc = ctx.enter_context(
            tc.tile_pool(name="psum_misc", bufs=1, space="PSUM"))

        # constants
        rep4_sb = singles.tile([BL, 128], f16)
        nc.sync.dma_start(rep4_sb, rep4)
        rep4t_sb = singles.tile([128, BL], f16)
        nc.sync.dma_start(rep4t_sb, rep4t)
        eye_sb = singles.tile([128, 128], f16)
        nc.sync.dma_start(eye_sb, eye)
        xd_sb = singles.tile([128, G, BL], f16)
        nc.sync.dma_start(xd_sb, xd)
        eps_sb = singles.tile([BL, 1], f32)
        nc.vector.memset(eps_sb, EPS)
        zero_sb = singles.tile([BL, 1], f32)
        nc.vector.memset(zero_sb, 0.0)
        zero128_sb = singles.tile([128, 1], f32)
        nc.vector.memset(zero128_sb, 0.0)

        # u_hat store: [p=(i4,b), h, j, d] fp16, one tile per h-chunk so
        # consumers can start before production finishes
        u_q = [singles.tile([128, HC, J, D], f16, tag=f"u{q}",
                            name=f"u{q}") for q in range(NQ)]

        def u_slice(hlo, n):
            q, off = divmod(hlo, HC)
            assert off + n <= HC
            return u_q[q][:, off:off + n, :, :]

        # ---- stage A+B: s0 accumulation + u_hat production, 2 DMA halves --
        psum_s0 = psum_misc.tile([BL, JD], f32, tag="sps")
        for half in range(2):
            g0 = GH * half
            wdh = halves.tile([128, GH, JD], f16, tag="wdh")
            nc.sync.dma_start(wdh, wd[:, g0:g0 + GH, :])
            xbh = halves.tile([128, GH, 128], f16, tag="xbh")
            nc.sync.dma_start(xbh, xblk[:, g0:g0 + GH, :])
            for gl in range(GH):
                g = g0 + gl
                nc.tensor.matmul(psum_s0, xd_sb[:, g, :], wdh[:, gl, :],
                                 start=(g == 0), stop=(g == G - 1))
                for ph in range(2):
                    pu = psum_prod.tile([128, 2, 512], f32, tag="pu")
                    for rr in range(2):
                        r = 2 * ph + rr
                        nc.tensor.matmul(
                            pu[:, rr, :JD],
                            xbh[32 * r:32 * (r + 1), gl, :],
                            wdh[32 * r:32 * (r + 1), gl, :],
                            start=True, stop=True,
                            tile_position=(32 * r, 0),
                        )
                    h0 = 4 * g + 2 * ph
                    dst = u_slice(h0, 2).rearrange("p h j d -> p h (j d)")
                    src = pu[:, :, :JD]
                    if g % 2 == 0:
                        nc.vector.tensor_copy(dst, src)
                    else:
                        nc.scalar.copy(dst, src)

        # ---- squash helper ----
        def squash(psum_s, alpha, name):
            """v = squash(alpha * psum_s); returns (v_f32 [BL,J,D], v_f16)."""
            ssq = smalls.tile([BL, J, D], f32, tag="ssq")
            nc.scalar.activation(
                ssq, psum_s.rearrange("b (j d) -> b j d", j=J),
                mybir.ActivationFunctionType.Square, bias=zero_sb[:],
                scale=alpha)
            t = smalls.tile([BL, J], f32, tag="sqn")
            nc.vector.tensor_reduce(t, ssq, axis=AX, op=ADD)
            u1 = smalls.tile([BL, J], f32, tag="u1")
            nc.vector.tensor_scalar_add(u1, t, 1.0)
            r1 = smalls.tile([BL, J], f32, tag="r1")
            nc.vector.reciprocal(r1, u1)
            u2 = smalls.tile([BL, J], f32, tag="u2")
            nc.scalar.activation(u2, t, mybir.ActivationFunctionType.Sqrt,
                                 bias=eps_sb[:], scale=1.0)
            r2 = smalls.tile([BL, J], f32, tag="r2")
            nc.vector.reciprocal(r2, u2)
            f0 = smalls.tile([BL, J], f32, tag="f0")
            nc.vector.tensor_tensor(f0, t, r1, MUL)
            f1 = smalls.tile([BL, J], f32, tag="f1")
            nc.vector.tensor_tensor(f1, f0, r2, MUL)
            f2 = smalls.tile([BL, J], f32, tag="f2")
            nc.vector.tensor_scalar_mul(f2, f1, alpha)
            v = smalls.tile([BL, J, D], f32, tag="v" + name)
            nc.vector.tensor_tensor(
                v, psum_s.rearrange("b (j d) -> b j d", j=J),
                f2[:, :, None].to_broadcast([BL, J, D]), MUL)
            v16 = smalls.tile([BL, JD], f16, tag="v16" + name)
            nc.vector.tensor_copy(v16, v.rearrange("b j d -> b (j d)"))
            return v, v16

        def vrep(v16, name):
            """replicate [BL, JD] across i4 -> SBUF fp16 [128, J, D]."""
            pv = psum_misc.tile([128, JD], f32, tag="pvrep")
            nc.tensor.matmul(pv, rep4_sb, v16, start=True, stop=True)
            out = smalls.tile([128, J, D], f16, tag="vrep" + name)
            nc.vector.tensor_copy(out, pv.rearrange("p (j d) -> p j d", j=J))
            return out

        # cc-dot chunk: cc_psum[128, HC*J] = sum_d u*vr (+ carry), as
        # DVE mul + t8 tree level, then 8 accumulating identity matmuls.
        def cc_dot_chunk(q, vr, carry):
            hs = slice(q * HC, (q + 1) * HC)
            eng = nc.gpsimd if q == NQ - 1 else nc.vector
            cu = work.tile([128, HC, J, D], f16, tag="cu")
            eng.tensor_tensor(
                cu, u_q[q],
                vr[:, None, :, :].to_broadcast([128, HC, J, D]), MUL)
            t8 = work.tile([128, HC, J, 8], f16, tag="t8")
            nc.vector.tensor_tensor(t8, cu[:, :, :, 0:8], cu[:, :, :, 8:16],
                                    ADD)
            pc = psum_cc.tile([128, HC * J], f32, tag="pcc")
            for dd in range(8):
                nc.tensor.matmul(
                    pc, eye_sb, t8[:, :, :, dd],
                    start=(dd == 0), stop=(dd == 7 and carry is None))
            if carry is not None:
                nc.tensor.matmul(
                    pc, eye_sb, carry[:, hs, :],
                    start=False, stop=True)
            return pc

        # softmax chunk: pc [128, HC*J] -> c_pk chunk [128, HC, J, 2] fp16
        def softmax_chunk(q, pc, c_pk, cc_save):
            hs = slice(q * HC, (q + 1) * HC)
            pcv = pc.rearrange("p (h j) -> p h j", j=J)
            if cc_save is not None:
                nc.scalar.copy(cc_save[:, hs, :], pcv)
            ex = smalls.tile([128, HC, J], f16, tag="ex")
            nc.scalar.activation(ex, pcv, mybir.ActivationFunctionType.Exp,
                                 bias=zero128_sb[:])
            z = smalls.tile([128, HC], f32, tag="z")
            nc.vector.tensor_reduce(z, ex, axis=AX, op=ADD)
            rz = smalls.tile([128, HC], f32, tag="rz")
            nc.vector.reciprocal(rz, z)
            nc.vector.tensor_tensor(
                c_pk[:, hs, :, 0], ex,
                rz[:, :, None].to_broadcast([128, HC, J]), MUL)
            nc.gpsimd.tensor_tensor(
                c_pk[:, hs, :, 1], ex,
                rz[:, :, None].to_broadcast([128, HC, J]), MUL)

        # weighted-sum chunk: DVE 2x mul via packed c pairs, then per-h
        # accumulating matmuls into ps [BL, JD].
        def wsum_chunk(q, c_pk, ps, first, last):
            hs = slice(q * HC, (q + 1) * HC)
            eng = nc.gpsimd if q == NQ - 1 else nc.vector
            cs = work.tile([128, HC, J, 8, 2], f16, tag="cu")
            eng.tensor_tensor(
                cs, u_q[q].rearrange("p h j (a b) -> p h j a b", b=2),
                c_pk[:, hs, :, None, :].to_broadcast([128, HC, J, 8, 2]),
                MUL)
            csf = cs.rearrange("p h j a b -> p h (j a b)")
            for hh in range(HC):
                nc.tensor.matmul(ps, rep4t_sb, csf[:, hh, :],
                                 start=(first and hh == 0),
                                 stop=(last and hh == HC - 1))

        # ---- round 0: v0 from s0; cc1 = u.v0 ----
        v0, v0_16 = squash(psum_s0, 0.1, "0")
        vr0 = vrep(v0_16, "0")
        cc1_sb = singles.tile([128, H, J], f16)
        c_pk = singles.tile([128, H, J, 2], f16)

        # round-0 cc-dot + round-1 softmax, chunk-pipelined
        for q in range(NQ):
            pc = cc_dot_chunk(q, vr0, None)
            softmax_chunk(q, pc, c_pk, cc1_sb)

        # ---- round 1: s1 from c1; v1; cc2 = cc1 + u.v1 ----
        ps1 = psum_misc.tile([BL, JD], f32, tag="sps")
        for q in range(NQ):
            wsum_chunk(q, c_pk, ps1, q == 0, q == NQ - 1)
        v1, v1_16 = squash(ps1, 1.0, "1")
        vr1 = vrep(v1_16, "1")
        for q in range(NQ):
            pc = cc_dot_chunk(q, vr1, cc1_sb)
            softmax_chunk(q, pc, c_pk, None)

        # ---- round 2: s2 from c2; v2 out ----
        ps2 = psum_misc.tile([BL, JD], f32, tag="sps")
        for q in range(NQ):
            wsum_chunk(q, c_pk, ps2, q == 0, q == NQ - 1)
        v2, _ = squash(ps2, 1.0, "2")
        nc.sync.dma_start(vout, v2.rearrange("b j d -> b (j d)"))

    nc.compile()
    return nc


_NC_CACHE = None


def _get_nc():
    global _NC_CACHE
    if _NC_CACHE is None:
        _NC_CACHE = _build_bass()
    return _NC_CACHE


def _prep_inputs(inputs, W):
    """Host-side layout marshaling -> per-core in_maps."""
    W = np.asarray(W, np.float32)
    x = np.asarray(inputs, np.float32)
    # wd[8*i16+e, g, 16j+d] = W[j, 16g+i16, d, e]
    wd = np.ascontiguousarray(
        W.reshape(J, G, 16, D, E).transpose(2, 4, 1, 0, 3)
        .reshape(128, G, JD)).astype(np.float16)
    rep4 = np.tile(np.eye(BL, dtype=np.float16), (1, 4))          # [32,128]
    rep4t = np.ascontiguousarray(rep4.T)                          # [128,32]
    eye = np.eye(128, dtype=np.float16)

    in_maps = []
    for cid in range(NCORES):
        xb = x[cid * BL:(cid + 1) * BL]                           # [32,1152,8]
        # xd[8*i16+e, g, b]
        xdc = np.ascontiguousarray(
            xb.reshape(BL, G, 16, E).transpose(2, 3, 1, 0)
            .reshape(128, G, BL)).astype(np.float16)
        # xblk[32r+8c+e, g, 32c+b] block-diagonal
        xr2 = xb.reshape(BL, G, 4, 4, E).transpose(2, 3, 4, 1, 0)  # r c e g b
        z = np.zeros((4, 4, E, G, 4, BL), np.float16)
        for c4 in range(4):
            z[:, c4, :, :, c4, :] = xr2[:, c4, :, :, :]
        xblkc = np.ascontiguousarray(
            z.transpose(0, 1, 2, 3, 4, 5).reshape(128, G, 128))
        in_maps.append({
            "xd": xdc, "xblk": xblkc, "wd": wd,
            "rep4": rep4, "rep4t": rep4t, "eye": eye,
        })
    return in_maps


def _run(inputs, W, trace=False):
    nc = _get_nc()
    in_maps = _prep_inputs(inputs, W)
    res = run_bass_kernel_spmd(nc, in_maps, core_ids=list(range(NCORES)),
                               trace=trace)
    out = np.concatenate([r["vout"] for r in res.results], axis=0)
    return out.reshape(B, J, D).astype(np.float32), res


def kernel(inputs, W):
    out, _ = _run(inputs, W, trace=False)
    return out


# revision 21
# speedup vs baseline: 1.1556x; 1.1556x over previous
"""CapsNet dynamic-routing layer on 8 Trainium2 NeuronCores.

reference: u_hat = einsum('jide,bie->bjid', W, x); 3 rounds of
softmax(cc, axis=j) -> s = sum_i c*u_hat -> v = squash(s) -> cc += u_hat.v

Strategy (pure data parallel, batch 256 -> 32 per core, W replicated):
  - u_hat produced on the PE with block-diagonal x stationaries
    [K=32=(i4,e8), M=128=(i4,b32)] x dense W slices [K=32, N=160=(j,d)],
    4 row-tiles (tile_position) in flight -> PSUM [(i4,b32), (j,d)],
    evacuated (f32->fp16, DVE/ACT split) to SBUF u[128, h=288, j, d],
    i = 4*h + i4.
  - round 0: c uniform -> s0 = (1/10) sum_i u_hat via dense K=128 matmuls
    accumulating 72 tiles into one PSUM bank.
  - cc-dot (cc += u.v): DVE mul (2x) + one halving tree level, then the
    d-reduction and the cross-partition i4-reduction ride the PE as
    accumulating matmuls with an identity stationary into PSUM chunks.
  - softmax consumes cc PSUM chunks directly (exp on ACT); c is stored as
    duplicated fp16 pairs (c_pk[..., 2]) so the weighted-sum multiply
    broadcasts c over d via packed 32-bit reads at DVE 2x.
  - weighted sum: DVE mul (2x) + per-h accumulating PE matmuls with a
    0/1 replication stationary -> s in PSUM [32, (j,d)]; squash in f32.
All host-side work is layout marshaling only (scatter/transpose/cast).
"""

import sys
from contextlib import ExitStack

import numpy as np

sys.path.insert(0, "/opt/trn_rl_repo")

import concourse.bacc as bacc  # noqa: E402
import concourse.tile as tile  # noqa: E402
from concourse import mybir  # noqa: E402
from concourse.bass_utils import run_bass_kernel_spmd  # noqa: E402

B, J, I, D, E = 256, 10, 1152, 16, 8
NCORES = 8
BL = B // NCORES            # 32 local batch per core
G = I // 16                 # 72 groups of 16 input caps
GH = G // 2                 # 36 groups per DMA half
H = I // 4                  # 288 (h = 4*g + r), i = 4*h + c4
JD = J * D                  # 160
EPS = 1e-7
HC = 48                     # h-chunk for routing passes
NQ = H // HC                # 6 chunks

f32 = mybir.dt.float32
f16 = mybir.dt.float16
ADD = mybir.AluOpType.add
MUL = mybir.AluOpType.mult
AX = mybir.AxisListType.X


def _build_bass():
    nc = bacc.Bacc("TRN2", target_bir_lowering=False)

    xd = nc.dram_tensor("xd", [128, G, BL], f16, kind="ExternalInput").ap()
    xblk = nc.dram_tensor("xblk", [128, G, 128], f16,
                          kind="ExternalInput").ap()
    wd = nc.dram_tensor("wd", [128, G, JD], f16, kind="ExternalInput").ap()
    rep4 = nc.dram_tensor("rep4", [BL, 128], f16, kind="ExternalInput").ap()
    rep4t = nc.dram_tensor("rep4t", [128, BL], f16, kind="ExternalInput").ap()
    eye = nc.dram_tensor("eye", [128, 128], f16, kind="ExternalInput").ap()
    vout = nc.dram_tensor("vout", [BL, JD], f32, kind="ExternalOutput").ap()

    with tile.TileContext(nc) as tc, ExitStack() as ctx, \
            nc.allow_low_precision("fp16 intermediates within tolerance"):
        singles = ctx.enter_context(tc.tile_pool(name="singles", bufs=1))
        halves = ctx.enter_context(tc.tile_pool(name="halves", bufs=2))
        work = ctx.enter_context(tc.tile_pool(name="work", bufs=2))
        smalls = ctx.enter_context(tc.tile_pool(name="smalls", bufs=1))
        psum_prod = ctx.enter_context(
            tc.tile_pool(name="psum_prod", bufs=2, space="PSUM"))
        psum_cc = ctx.enter_context(
            tc.tile_pool(name="psum_cc", bufs=2, space="PSUM"))
        psum_misc = ctx.enter_context(
            tc.tile_pool(name="psum_misc", bufs=1, space="PSUM"))

        # constants
        rep4_sb = singles.tile([BL, 128], f16)
        nc.sync.dma_start(rep4_sb, rep4)
        rep4t_sb = singles.tile([128, BL], f16)
        nc.sync.dma_start(rep4t_sb, rep4t)
        eye_sb = singles.tile([128, 128], f16)
        nc.sync.dma_start(eye_sb, eye)
        xd_sb = singles.tile([128, G, BL], f16)
        nc.sync.dma_start(xd_sb, xd)
        eps_sb = singles.tile([BL, 1], f32)
        nc.vector.memset(eps_sb, EPS)
        zero_sb = singles.tile([BL, 1], f32)
        nc.vector.memset(zero_sb, 0.0)
        zero128_sb = singles.tile([128, 1], f32)
        nc.vector.memset(zero128_sb, 0.0)

        # u_hat store: [p=(i4,b), h, j, d] fp16, one tile per h-chunk so
        # consumers can start before production finishes
        u_q = [singles.tile([128, HC, J, D], f16, tag=f"u{q}",
                            name=f"u{q}") for q in range(NQ)]

        def u_slice(hlo, n):
            q, off = divmod(hlo, HC)
            assert off + n <= HC
            return u_q[q][:, off:off + n, :, :]

        # ---- stage A+B: s0 accumulation + u_hat production, 2 DMA halves --
        psum_s0 = psum_misc.tile([BL, JD], f32, tag="sps")
        for half in range(2):
            g0 = GH * half
            wdh = halves.tile([128, GH, JD], f16, tag="wdh")
            nc.sync.dma_start(wdh, wd[:, g0:g0 + GH, :])
            xbh = halves.tile([128, GH, 128], f16, tag="xbh")
            nc.sync.dma_start(xbh, xblk[:, g0:g0 + GH, :])
            for gl in range(GH):
                g = g0 + gl
                nc.tensor.matmul(psum_s0, xd_sb[:, g, :], wdh[:, gl, :],
                                 start=(g == 0), stop=(g == G - 1))
                for ph in range(2):
                    pu = psum_prod.tile([128, 2, 512], f32, tag="pu")
                    for rr in range(2):
                        r = 2 * ph + rr
                        nc.tensor.matmul(
                            pu[:, rr, :JD],
                            xbh[32 * r:32 * (r + 1), gl, :],
                            wdh[32 * r:32 * (r + 1), gl, :],
                            start=True, stop=True,
                            tile_position=(32 * r, 0),
                        )
                    h0 = 4 * g + 2 * ph
                    dst = u_slice(h0, 2).rearrange("p h j d -> p h (j d)")
                    src = pu[:, :, :JD]
                    if g % 2 == 0:
                        nc.vector.tensor_copy(dst, src)
                    else:
                        nc.scalar.copy(dst, src)

        # ---- squash helper ----
        def squash(psum_s, alpha, name):
            """v = squash(alpha * psum_s); returns (v_f32 [BL,J,D], v_f16)."""
            ssq = smalls.tile([BL, J, D], f32, tag="ssq")
            nc.scalar.activation(
                ssq, psum_s.rearrange("b (j d) -> b j d", j=J),
                mybir.ActivationFunctionType.Square, bias=zero_sb[:],
                scale=alpha)
            t = smalls.tile([BL, J], f32, tag="sqn")
            nc.vector.tensor_reduce(t, ssq, axis=AX, op=ADD)
            u1 = smalls.tile([BL, J], f32, tag="u1")
            nc.vector.tensor_scalar_add(u1, t, 1.0)
            r1 = smalls.tile([BL, J], f32, tag="r1")
            nc.vector.reciprocal(r1, u1)
            u2 = smalls.tile([BL, J], f32, tag="u2")
            nc.scalar.activation(u2, t, mybir.ActivationFunctionType.Sqrt,
                                 bias=eps_sb[:], scale=1.0)
            r2 = smalls.tile([BL, J], f32, tag="r2")
            nc.vector.reciprocal(r2, u2)
            f0 = smalls.tile([BL, J], f32, tag="f0")
            nc.vector.tensor_tensor(f0, t, r1, MUL)
            f1 = smalls.tile([BL, J], f32, tag="f1")
            nc.vector.tensor_tensor(f1, f0, r2, MUL)
            f2 = smalls.tile([BL, J], f32, tag="f2")
            nc.vector.tensor_scalar_mul(f2, f1, alpha)
            v = smalls.tile([BL, J, D], f32, tag="v" + name)
            nc.vector.tensor_tensor(
                v, psum_s.rearrange("b (j d) -> b j d", j=J),
                f2[:, :, None].to_broadcast([BL, J, D]), MUL)
            v16 = smalls.tile([BL, JD], f16, tag="v16" + name)
            nc.vector.tensor_copy(v16, v.rearrange("b j d -> b (j d)"))
            return v, v16

        def vrep(v16, name):
            """replicate [BL, JD] across i4 -> SBUF fp16 [128, J, D]."""
            pv = psum_misc.tile([128, JD], f32, tag="pvrep")
            nc.tensor.matmul(pv, rep4_sb, v16, start=True, stop=True)
            out = smalls.tile([128, J, D], f16, tag="vrep" + name)
            nc.vector.tensor_copy(out, pv.rearrange("p (j d) -> p j d", j=J))
            return out

        # cc-dot chunk: cc_psum[128, HC*J] = sum_d u*vr (+ carry), as
        # DVE mul + t8 tree level, then 8 accumulating identity matmuls.
        def cc_dot_chunk(q, vr, carry):
            hs = slice(q * HC, (q + 1) * HC)
            eng = nc.vector
            cu = work.tile([128, HC, J, D], f16, tag="cu")
            eng.tensor_tensor(
                cu, u_q[q],
                vr[:, None, :, :].to_broadcast([128, HC, J, D]), MUL)
            t8 = work.tile([128, HC, J, 8], f16, tag="t8")
            nc.vector.tensor_tensor(t8, cu[:, :, :, 0:8], cu[:, :, :, 8:16],
                                    ADD)
            pc = psum_cc.tile([128, HC * J], f32, tag="pcc")
            for dd in range(8):
                nc.tensor.matmul(
                    pc, eye_sb, t8[:, :, :, dd],
                    start=(dd == 0), stop=(dd == 7 and carry is None))
            if carry is not None:
                nc.tensor.matmul(
                    pc, eye_sb, carry[:, hs, :],
                    start=False, stop=True)
            return pc

        # softmax chunk: pc [128, HC*J] -> c_pk chunk [128, HC, J, 2] fp16
        def softmax_chunk(q, pc, c_pk, cc_save):
            hs = slice(q * HC, (q + 1) * HC)
            pcv = pc.rearrange("p (h j) -> p h j", j=J)
            if cc_save is not None:
                nc.scalar.copy(cc_save[:, hs, :], pcv)
            ex = smalls.tile([128, HC, J], f16, tag="ex")
            nc.scalar.activation(ex, pcv, mybir.ActivationFunctionType.Exp,
                                 bias=zero128_sb[:])
            z = smalls.tile([128, HC], f32, tag="z")
            nc.vector.tensor_reduce(z, ex, axis=AX, op=ADD)
            rz = smalls.tile([128, HC], f32, tag="rz")
            nc.vector.reciprocal(rz, z)
            nc.vector.tensor_tensor(
                c_pk[:, hs, :, 0], ex,
                rz[:, :, None].to_broadcast([128, HC, J]), MUL)
            nc.gpsimd.tensor_tensor(
                c_pk[:, hs, :, 1], ex,
                rz[:, :, None].to_broadcast([128, HC, J]), MUL)

        # weighted-sum chunk: DVE 2x mul via packed c pairs, then per-h
        # accumulating matmuls into ps [BL, JD].
        def wsum_chunk(q, c_pk, ps, first, last):
            hs = slice(q * HC, (q + 1) * HC)
            eng = nc.vector
            cs = work.tile([128, HC, J, 8, 2], f16, tag="cu")
            eng.tensor_tensor(
                cs, u_q[q].rearrange("p h j (a b) -> p h j a b", b=2),
                c_pk[:, hs, :, None, :].to_broadcast([128, HC, J, 8, 2]),
                MUL)
            csf = cs.rearrange("p h j a b -> p h (j a b)")
            for hh in range(HC):
                nc.tensor.matmul(ps, rep4t_sb, csf[:, hh, :],
                                 start=(first and hh == 0),
                                 stop=(last and hh == HC - 1))

        # ---- round 0: v0 from s0; cc1 = u.v0 ----
        v0, v0_16 = squash(psum_s0, 0.1, "0")
        vr0 = vrep(v0_16, "0")
        cc1_sb = singles.tile([128, H, J], f16)
        c_pk = singles.tile([128, H, J, 2], f16)

        # round-0 cc-dot + round-1 softmax, chunk-pipelined
        for q in range(NQ):
            pc = cc_dot_chunk(q, vr0, None)
            softmax_chunk(q, pc, c_pk, cc1_sb)

        # ---- round 1: s1 from c1; v1; cc2 = cc1 + u.v1 ----
        ps1 = psum_misc.tile([BL, JD], f32, tag="sps")
        for q in range(NQ):
            wsum_chunk(q, c_pk, ps1, q == 0, q == NQ - 1)
        v1, v1_16 = squash(ps1, 1.0, "1")
        vr1 = vrep(v1_16, "1")
        for q in range(NQ):
            pc = cc_dot_chunk(q, vr1, cc1_sb)
            softmax_chunk(q, pc, c_pk, None)

        # ---- round 2: s2 from c2; v2 out ----
        ps2 = psum_misc.tile([BL, JD], f32, tag="sps")
        for q in range(NQ):
            wsum_chunk(q, c_pk, ps2, q == 0, q == NQ - 1)
        v2, _ = squash(ps2, 1.0, "2")
        nc.sync.dma_start(vout, v2.rearrange("b j d -> b (j d)"))

    nc.compile()
    return nc


_NC_CACHE = None


def _get_nc():
    global _NC_CACHE
    if _NC_CACHE is None:
        _NC_CACHE = _build_bass()
    return _NC_CACHE


def _prep_inputs(inputs, W):
    """Host-side layout marshaling -> per-core in_maps."""
    W = np.asarray(W, np.float32)
    x = np.asarray(inputs, np.float32)
    # wd[8*i16+e, g, 16j+d] = W[j, 16g+i16, d, e]
    wd = np.ascontiguousarray(
        W.reshape(J, G, 16, D, E).transpose(2, 4, 1, 0, 3)
        .reshape(128, G, JD)).astype(np.float16)
    rep4 = np.tile(np.eye(BL, dtype=np.float16), (1, 4))          # [32,128]
    rep4t = np.ascontiguousarray(rep4.T)                          # [128,32]
    eye = np.eye(128, dtype=np.float16)

    in_maps = []
    for cid in range(NCORES):
        xb = x[cid * BL:(cid + 1) * BL]                           # [32,1152,8]
        # xd[8*i16+e, g, b]
        xdc = np.ascontiguousarray(
            xb.reshape(BL, G, 16, E).transpose(2, 3, 1, 0)
            .reshape(128, G, BL)).astype(np.float16)
        # xblk[32r+8c+e, g, 32c+b] block-diagonal
        xr2 = xb.reshape(BL, G, 4, 4, E).transpose(2, 3, 4, 1, 0)  # r c e g b
        z = np.zeros((4, 4, E, G, 4, BL), np.float16)
        for c4 in range(4):
            z[:, c4, :, :, c4, :] = xr2[:, c4, :, :, :]
        xblkc = np.ascontiguousarray(
            z.transpose(0, 1, 2, 3, 4, 5).reshape(128, G, 128))
        in_maps.append({
            "xd": xdc, "xblk": xblkc, "wd": wd,
            "rep4": rep4, "rep4t": rep4t, "eye": eye,
        })
    return in_maps


def _run(inputs, W, trace=False):
    nc = _get_nc()
    in_maps = _prep_inputs(inputs, W)
    res = run_bass_kernel_spmd(nc, in_maps, core_ids=list(range(NCORES)),
                               trace=trace)
    out = np.concatenate([r["vout"] for r in res.results], axis=0)
    return out.reshape(B, J, D).astype(np.float32), res


def kernel(inputs, W):
    out, _ = _run(inputs, W, trace=False)
    return out


# revision 26
# speedup vs baseline: 1.1893x; 1.0292x over previous
"""CapsNet dynamic-routing layer on 8 Trainium2 NeuronCores.

reference: u_hat = einsum('jide,bie->bjid', W, x); 3 rounds of
softmax(cc, axis=j) -> s = sum_i c*u_hat -> v = squash(s) -> cc += u_hat.v

Strategy (pure data parallel, batch 256 -> 32 per core, W replicated):
  - u_hat produced on the PE with block-diagonal x stationaries
    [K=32=(i4,e8), M=128=(i4,b32)] x dense W slices [K=32, N=160=(j,d)],
    4 row-tiles (tile_position) in flight -> PSUM [(i4,b32), (j,d)],
    evacuated (f32->fp16, DVE/ACT split) to SBUF u[128, h=288, j, d],
    i = 4*h + i4.
  - round 0: c uniform -> s0 = (1/10) sum_i u_hat via dense K=128 matmuls
    accumulating 72 tiles into one PSUM bank.
  - cc-dot (cc += u.v): DVE mul (2x) + one halving tree level, then the
    d-reduction and the cross-partition i4-reduction ride the PE as
    accumulating matmuls with an identity stationary into PSUM chunks.
  - softmax consumes cc PSUM chunks directly (exp on ACT); c is stored as
    duplicated fp16 pairs (c_pk[..., 2]) so the weighted-sum multiply
    broadcasts c over d via packed 32-bit reads at DVE 2x.
  - weighted sum: DVE mul (2x) + per-h accumulating PE matmuls with a
    0/1 replication stationary -> s in PSUM [32, (j,d)]; squash in f32.
All host-side work is layout marshaling only (scatter/transpose/cast).
"""

import sys
from contextlib import ExitStack

import numpy as np

sys.path.insert(0, "/opt/trn_rl_repo")

import concourse.bacc as bacc  # noqa: E402
import concourse.tile as tile  # noqa: E402
from concourse import mybir  # noqa: E402
from concourse.bass_utils import run_bass_kernel_spmd  # noqa: E402

B, J, I, D, E = 256, 10, 1152, 16, 8
NCORES = 8
BL = B // NCORES            # 32 local batch per core
G = I // 16                 # 72 groups of 16 input caps
GH = G // 2                 # 36 groups per DMA half
H = I // 4                  # 288 (h = 4*g + r), i = 4*h + c4
JD = J * D                  # 160
EPS = 1e-7
HC = 48                     # h-chunk for routing passes
NQ = H // HC                # 6 chunks

f32 = mybir.dt.float32
f16 = mybir.dt.float16
ADD = mybir.AluOpType.add
MUL = mybir.AluOpType.mult
AX = mybir.AxisListType.X


def _build_bass():
    nc = bacc.Bacc("TRN2", target_bir_lowering=False)

    xd = nc.dram_tensor("xd", [128, G, BL], f16, kind="ExternalInput").ap()
    xblk = nc.dram_tensor("xblk", [128, G, 128], f16,
                          kind="ExternalInput").ap()
    wd = nc.dram_tensor("wd", [128, G, JD], f16, kind="ExternalInput").ap()
    rep4 = nc.dram_tensor("rep4", [BL, 128], f16, kind="ExternalInput").ap()
    rep4t = nc.dram_tensor("rep4t", [128, BL], f16, kind="ExternalInput").ap()
    eye = nc.dram_tensor("eye", [128, 128], f16, kind="ExternalInput").ap()
    vout = nc.dram_tensor("vout", [BL, JD], f32, kind="ExternalOutput").ap()

    with tile.TileContext(nc) as tc, ExitStack() as ctx, \
            nc.allow_low_precision("fp16 intermediates within tolerance"):
        singles = ctx.enter_context(tc.tile_pool(name="singles", bufs=1))
        halves = ctx.enter_context(tc.tile_pool(name="halves", bufs=2))
        work = ctx.enter_context(tc.tile_pool(name="work", bufs=2))
        smalls = ctx.enter_context(tc.tile_pool(name="smalls", bufs=1))
        psum_prod = ctx.enter_context(
            tc.tile_pool(name="psum_prod", bufs=2, space="PSUM"))
        psum_cc = ctx.enter_context(
            tc.tile_pool(name="psum_cc", bufs=2, space="PSUM"))
        psum_misc = ctx.enter_context(
            tc.tile_pool(name="psum_misc", bufs=1, space="PSUM"))

        # constants
        rep4_sb = singles.tile([BL, 128], f16)
        nc.sync.dma_start(rep4_sb, rep4)
        rep4t_sb = singles.tile([128, BL], f16)
        nc.sync.dma_start(rep4t_sb, rep4t)
        eye_sb = singles.tile([128, 128], f16)
        nc.sync.dma_start(eye_sb, eye)
        xd_sb = singles.tile([128, G, BL], f16)
        nc.sync.dma_start(xd_sb, xd)
        eps_sb = singles.tile([BL, 1], f32)
        nc.vector.memset(eps_sb, EPS)
        zero_sb = singles.tile([BL, 1], f32)
        nc.vector.memset(zero_sb, 0.0)
        zero128_sb = singles.tile([128, 1], f32)
        nc.vector.memset(zero128_sb, 0.0)

        # u_hat store: [p=(i4,b), h, j, d] fp16, one tile per h-chunk so
        # consumers can start before production finishes
        u_q = [singles.tile([128, HC, J, D], f16, tag=f"u{q}",
                            name=f"u{q}") for q in range(NQ)]

        def u_slice(hlo, n):
            q, off = divmod(hlo, HC)
            assert off + n <= HC
            return u_q[q][:, off:off + n, :, :]

        # ---- stage A+B: s0 accumulation + u_hat production, 2 DMA halves --
        psum_s0 = psum_misc.tile([BL, JD], f32, tag="sps")
        for half in range(2):
            g0 = GH * half
            wdh = halves.tile([128, GH, JD], f16, tag="wdh")
            nc.sync.dma_start(wdh, wd[:, g0:g0 + GH, :])
            xbh = halves.tile([128, GH, 128], f16, tag="xbh")
            nc.sync.dma_start(xbh, xblk[:, g0:g0 + GH, :])
            for gl in range(GH):
                g = g0 + gl
                nc.tensor.matmul(psum_s0, xd_sb[:, g, :], wdh[:, gl, :],
                                 start=(g == 0), stop=(g == G - 1))
                for ph in range(2):
                    pu = psum_prod.tile([128, 2, 512], f32, tag="pu")
                    for rr in range(2):
                        r = 2 * ph + rr
                        nc.tensor.matmul(
                            pu[:, rr, :JD],
                            xbh[32 * r:32 * (r + 1), gl, :],
                            wdh[32 * r:32 * (r + 1), gl, :],
                            start=True, stop=True,
                            tile_position=(32 * r, 0),
                        )
                    h0 = 4 * g + 2 * ph
                    dst = u_slice(h0, 2).rearrange("p h j d -> p h (j d)")
                    src = pu[:, :, :JD]
                    if g % 2 == 0:
                        nc.vector.tensor_copy(dst, src)
                    else:
                        nc.scalar.copy(dst, src)

        # ---- squash helper ----
        def squash(psum_s, alpha, name):
            """v = squash(alpha * psum_s); returns (v_f32 [BL,J,D], v_f16)."""
            ssq = smalls.tile([BL, J, D], f32, tag="ssq")
            nc.scalar.activation(
                ssq, psum_s.rearrange("b (j d) -> b j d", j=J),
                mybir.ActivationFunctionType.Square, bias=zero_sb[:],
                scale=alpha)
            t = smalls.tile([BL, J], f32, tag="sqn")
            nc.vector.tensor_reduce(t, ssq, axis=AX, op=ADD)
            u1 = smalls.tile([BL, J], f32, tag="u1")
            nc.vector.tensor_scalar_add(u1, t, 1.0)
            r1 = smalls.tile([BL, J], f32, tag="r1")
            nc.vector.reciprocal(r1, u1)
            u2 = smalls.tile([BL, J], f32, tag="u2")
            nc.scalar.activation(u2, t, mybir.ActivationFunctionType.Sqrt,
                                 bias=eps_sb[:], scale=1.0)
            r2 = smalls.tile([BL, J], f32, tag="r2")
            nc.vector.reciprocal(r2, u2)
            f0 = smalls.tile([BL, J], f32, tag="f0")
            nc.vector.tensor_tensor(f0, t, r1, MUL)
            f1 = smalls.tile([BL, J], f32, tag="f1")
            nc.vector.tensor_tensor(f1, f0, r2, MUL)
            f2 = smalls.tile([BL, J], f32, tag="f2")
            nc.vector.tensor_scalar_mul(f2, f1, alpha)
            v = smalls.tile([BL, J, D], f32, tag="v" + name)
            nc.vector.tensor_tensor(
                v, psum_s.rearrange("b (j d) -> b j d", j=J),
                f2[:, :, None].to_broadcast([BL, J, D]), MUL)
            v16 = smalls.tile([BL, JD], f16, tag="v16" + name)
            nc.vector.tensor_copy(v16, v.rearrange("b j d -> b (j d)"))
            return v, v16

        def vrep(v16, name):
            """replicate [BL, JD] across i4 -> SBUF fp16 [128, J, D]."""
            pv = psum_misc.tile([128, JD], f32, tag="pvrep")
            nc.tensor.matmul(pv, rep4_sb, v16, start=True, stop=True)
            out = smalls.tile([128, J, D], f16, tag="vrep" + name)
            nc.vector.tensor_copy(out, pv.rearrange("p (j d) -> p j d", j=J))
            return out

        # cc-dot chunk: cc_psum[128, HC*J] = sum_d u*vr (+ carry), as
        # DVE mul + t8 tree level, then 8 accumulating identity matmuls.
        def cc_dot_chunk(q, vr, carry):
            hs = slice(q * HC, (q + 1) * HC)
            eng = nc.vector
            cu = work.tile([128, HC, J, D], f16, tag="cu")
            eng.tensor_tensor(
                cu, u_q[q],
                vr[:, None, :, :].to_broadcast([128, HC, J, D]), MUL)
            t8 = work.tile([128, HC, J, 8], f16, tag="t8")
            nc.vector.tensor_tensor(t8, cu[:, :, :, 0:8], cu[:, :, :, 8:16],
                                    ADD)
            pc = psum_cc.tile([128, HC * J], f32, tag="pcc")
            for dd in range(8):
                nc.tensor.matmul(
                    pc, eye_sb, t8[:, :, :, dd],
                    start=(dd == 0), stop=(dd == 7 and carry is None))
            if carry is not None:
                nc.tensor.matmul(
                    pc, eye_sb, carry[:, hs, :],
                    start=False, stop=True)
            return pc

        # softmax chunk: pc [128, HC*J] -> c_pk chunk [128, HC, J, 2] fp16
        def softmax_chunk(q, pc, c_pk, cc_save):
            hs = slice(q * HC, (q + 1) * HC)
            pcv = pc.rearrange("p (h j) -> p h j", j=J)
            if cc_save is not None:
                nc.scalar.copy(cc_save[:, hs, :], pcv)
            ex = smalls.tile([128, HC, J], f16, tag="ex")
            nc.scalar.activation(ex, pcv, mybir.ActivationFunctionType.Exp,
                                 bias=zero128_sb[:])
            z = smalls.tile([128, HC], f32, tag="z")
            nc.vector.tensor_reduce(z, ex, axis=AX, op=ADD)
            rz = smalls.tile([128, HC], f32, tag="rz")
            nc.vector.reciprocal(rz, z)
            nc.gpsimd.tensor_tensor(
                c_pk[:, hs, :, 0], ex,
                rz[:, :, None].to_broadcast([128, HC, J]), MUL)
            nc.gpsimd.tensor_tensor(
                c_pk[:, hs, :, 1], ex,
                rz[:, :, None].to_broadcast([128, HC, J]), MUL)

        # weighted-sum chunk: DVE 2x mul via packed c pairs, then per-h
        # accumulating matmuls into ps [BL, JD].
        def wsum_chunk(q, c_pk, ps, first, last):
            hs = slice(q * HC, (q + 1) * HC)
            eng = nc.vector
            cs = work.tile([128, HC, J, 8, 2], f16, tag="cu")
            eng.tensor_tensor(
                cs, u_q[q].rearrange("p h j (a b) -> p h j a b", b=2),
                c_pk[:, hs, :, None, :].to_broadcast([128, HC, J, 8, 2]),
                MUL)
            csf = cs.rearrange("p h j a b -> p h (j a b)")
            for hh in range(HC):
                nc.tensor.matmul(ps, rep4t_sb, csf[:, hh, :],
                                 start=(first and hh == 0),
                                 stop=(last and hh == HC - 1))

        # ---- round 0: v0 from s0; cc1 = u.v0 ----
        v0, v0_16 = squash(psum_s0, 0.1, "0")
        vr0 = vrep(v0_16, "0")
        cc1_sb = singles.tile([128, H, J], f16)
        c_pk = singles.tile([128, H, J, 2], f16)

        # round-0 cc-dot + round-1 softmax, chunk-pipelined
        for q in range(NQ):
            pc = cc_dot_chunk(q, vr0, None)
            softmax_chunk(q, pc, c_pk, cc1_sb)

        # ---- round 1: s1 from c1; v1; cc2 = cc1 + u.v1 ----
        ps1 = psum_misc.tile([BL, JD], f32, tag="sps")
        for q in range(NQ):
            wsum_chunk(q, c_pk, ps1, q == 0, q == NQ - 1)
        v1, v1_16 = squash(ps1, 1.0, "1")
        vr1 = vrep(v1_16, "1")
        for q in range(NQ):
            pc = cc_dot_chunk(q, vr1, cc1_sb)
            softmax_chunk(q, pc, c_pk, None)

        # ---- round 2: s2 from c2; v2 out ----
        ps2 = psum_misc.tile([BL, JD], f32, tag="sps")
        for q in range(NQ):
            wsum_chunk(q, c_pk, ps2, q == 0, q == NQ - 1)
        v2, _ = squash(ps2, 1.0, "2")
        nc.sync.dma_start(vout, v2.rearrange("b j d -> b (j d)"))

    nc.compile()
    return nc


_NC_CACHE = None


def _get_nc():
    global _NC_CACHE
    if _NC_CACHE is None:
        _NC_CACHE = _build_bass()
    return _NC_CACHE


def _prep_inputs(inputs, W):
    """Host-side layout marshaling -> per-core in_maps."""
    W = np.asarray(W, np.float32)
    x = np.asarray(inputs, np.float32)
    # wd[8*i16+e, g, 16j+d] = W[j, 16g+i16, d, e]
    wd = np.ascontiguousarray(
        W.reshape(J, G, 16, D, E).transpose(2, 4, 1, 0, 3)
        .reshape(128, G, JD)).astype(np.float16)
    rep4 = np.tile(np.eye(BL, dtype=np.float16), (1, 4))          # [32,128]
    rep4t = np.ascontiguousarray(rep4.T)                          # [128,32]
    eye = np.eye(128, dtype=np.float16)

    in_maps = []
    for cid in range(NCORES):
        xb = x[cid * BL:(cid + 1) * BL]                           # [32,1152,8]
        # xd[8*i16+e, g, b]
        xdc = np.ascontiguousarray(
            xb.reshape(BL, G, 16, E).transpose(2, 3, 1, 0)
            .reshape(128, G, BL)).astype(np.float16)
        # xblk[32r+8c+e, g, 32c+b] block-diagonal
        xr2 = xb.reshape(BL, G, 4, 4, E).transpose(2, 3, 4, 1, 0)  # r c e g b
        z = np.zeros((4, 4, E, G, 4, BL), np.float16)
        for c4 in range(4):
            z[:, c4, :, :, c4, :] = xr2[:, c4, :, :, :]
        xblkc = np.ascontiguousarray(
            z.transpose(0, 1, 2, 3, 4, 5).reshape(128, G, 128))
        in_maps.append({
            "xd": xdc, "xblk": xblkc, "wd": wd,
            "rep4": rep4, "rep4t": rep4t, "eye": eye,
        })
    return in_maps


def _run(inputs, W, trace=False):
    nc = _get_nc()
    in_maps = _prep_inputs(inputs, W)
    res = run_bass_kernel_spmd(nc, in_maps, core_ids=list(range(NCORES)),
                               trace=trace)
    out = np.concatenate([r["vout"] for r in res.results], axis=0)
    return out.reshape(B, J, D).astype(np.float32), res


def kernel(inputs, W):
    out, _ = _run(inputs, W, trace=False)
    return out
